# revision 12
# baseline (speedup 1.0000x reference)
"""Trainium2 Bass kernel for LocalAttention (nn_LocalAttention_86517821216554).

Reference computation (per batch n of N=8, data-parallel over 8 cores):
  q = query @ Wq.T + bq ; k = key @ Wk.T + bk          (E=512, H=8, Dh=64)
  s_h = (q_h @ k_h.T) / sqrt(Dh)                        per head, [L=1024, L]
  p_h = softmax(s_h, axis=-1)                           full-row softmax
  A   = sum_h p_h                                       (= 8 * w_avg)
  band(i) = [i-128, i+128] clipped to [0, L)
  w_loc[i, j] = A[i,j]*band_mask / (sum_{j in band} A[i,j'] + 8e-6)
  out[i, :]   = sum_j w_loc[i, j] * value[j, :]

Device kernel computes, per core, the band-packed w_loc (8 tiles of
[128, 384] windows) and out [1024, 512]; the host scatters the band into
the dense [1024, 1024] w_loc (exact zeros outside the band) and stacks
batches.

Matmuls run in bf16 (fp32 accumulate); softmax + renormalization in fp32.
"""

import sys

import numpy as np

for _p in ("/opt/trn_rl_repo",):
    if _p not in sys.path:
        sys.path.append(_p)

import ml_dtypes

L = 1024
N = 8
E = 512
H = 8
DH = 64
W2 = 128  # WINDOW // 2
WIN = 384  # per-i-tile column window: 256 + 128
T = L // 128  # 8 row tiles of 128
NCORES = 8

BF16 = ml_dtypes.bfloat16

_CACHE = {}


def _win_start(t: int) -> int:
    return min(max(128 * t - W2, 0), L - WIN)


def _make_masks() -> np.ndarray:
    """masks[sel, r, c] for the three distinct band patterns.

    sel 0 (t=0):    band c <= r + 128
    sel 1 (t=1..6): band r <= c <= r + 256
    sel 2 (t=7):    band c >= r + 128
    """
    r = np.arange(128)[:, None]
    c = np.arange(WIN)[None, :]
    m0 = (c <= r + W2).astype(np.float32)
    mm = ((c >= r) & (c <= r + 2 * W2)).astype(np.float32)
    m7 = (c >= r + W2).astype(np.float32)
    return np.stack([m0, mm, m7])


def _mask_sel(t: int) -> int:
    return 0 if t == 0 else (2 if t == T - 1 else 1)


def _build_program(use_bias: bool = False, phase: str = "full"):
    import concourse.bass as bass
    import concourse.tile as tile
    from concourse import bacc, mybir
    from contextlib import ExitStack

    f32 = mybir.dt.float32
    bf16 = mybir.dt.bfloat16
    Alu = mybir.AluOpType
    Act = mybir.ActivationFunctionType
    ts = bass.ts

    nc = bacc.Bacc("TRN2", target_bir_lowering=False, debug=False)

    # --- DRAM I/O ---------------------------------------------------------
    qT_d = nc.dram_tensor("qT", [E, L], bf16, kind="ExternalInput").ap()
    kT_d = nc.dram_tensor("kT", [E, L], bf16, kind="ExternalInput").ap()
    v_d = nc.dram_tensor("v", [L, E], bf16, kind="ExternalInput").ap()
    wqT_d = nc.dram_tensor("wqT", [E, E], bf16, kind="ExternalInput").ap()
    wkT_d = nc.dram_tensor("wkT", [E, E], bf16, kind="ExternalInput").ap()
    bq_d = nc.dram_tensor("bq", [128, 4], f32, kind="ExternalInput").ap()
    bk_d = nc.dram_tensor("bk", [128, 4], f32, kind="ExternalInput").ap()
    masks_d = nc.dram_tensor("masks", [3, 128, WIN], f32, kind="ExternalInput").ap()
    ident_d = nc.dram_tensor("ident", [128, 128], f32, kind="ExternalInput").ap()
    out_d = nc.dram_tensor("out", [L, E], f32, kind="ExternalOutput").ap()
    wband_d = nc.dram_tensor("wband", [T, 128, WIN], f32, kind="ExternalOutput").ap()

    with tile.TileContext(nc) as tc, ExitStack() as ctx:
        singles = ctx.enter_context(tc.tile_pool(name="singles", bufs=1))

        # persistent SBUF tiles (distinct tags so each gets its own slot)
        qT_sb = [singles.tile([128, L], bf16, tag=f"qT{i}", name=f"qT{i}") for i in range(4)]
        kT_sb = [singles.tile([128, L], bf16, tag=f"kT{i}", name=f"kT{i}") for i in range(4)]
        v_sb = [singles.tile([128, E], bf16, tag=f"v{i}", name=f"v{i}") for i in range(T)]
        wqT_sb = [singles.tile([128, E], bf16, tag=f"wqT{i}", name=f"wqT{i}") for i in range(4)]
        wkT_sb = [singles.tile([128, E], bf16, tag=f"wkT{i}", name=f"wkT{i}") for i in range(4)]
        mask_sb = [singles.tile([128, WIN], f32, tag=f"mask{i}", name=f"mask{i}") for i in range(3)]
        ident_sb = singles.tile([128, 128], f32, tag="ident", name="ident")
        bq_sb = singles.tile([128, 4], f32, tag="bq", name="bq_sb")
        bk_sb = singles.tile([128, 4], f32, tag="bk", name="bk_sb")
        qTp_sb = [singles.tile([128, L], bf16, tag=f"qTp{i}", name=f"qTp{i}") for i in range(4)]
        kTp_sb = [singles.tile([128, L], bf16, tag=f"kTp{i}", name=f"kTp{i}") for i in range(4)]

        for i in range(4):
            nc.sync.dma_start(out=qT_sb[i], in_=qT_d[ts(i, 128), :])
            nc.sync.dma_start(out=kT_sb[i], in_=kT_d[ts(i, 128), :])
            nc.sync.dma_start(out=wqT_sb[i], in_=wqT_d[ts(i, 128), :])
            nc.sync.dma_start(out=wkT_sb[i], in_=wkT_d[ts(i, 128), :])
        for i in range(T):
            nc.sync.dma_start(out=v_sb[i], in_=v_d[ts(i, 128), :])
        for i in range(3):
            nc.sync.dma_start(out=mask_sb[i], in_=masks_d[i])
        nc.sync.dma_start(out=ident_sb, in_=ident_d)
        nc.sync.dma_start(out=bq_sb, in_=bq_d)
        nc.sync.dma_start(out=bk_sb, in_=bk_d)

        # --- projections: xTp[e', i] = sum_e W.T[e, e'] * xT[e, i] + b ---
        with tc.tile_pool(name="proj_ps", bufs=2, space="PSUM") as proj_ps:
            for eb in range(4):
                for w_sb, x_sb, b_sb, o_sb in (
                    (wkT_sb, kT_sb, bk_sb, kTp_sb),
                    (wqT_sb, qT_sb, bq_sb, qTp_sb),
                ):
                    ps = proj_ps.tile([128, L], f32, tag="proj", name="proj_ps_t")
                    for ih in range(2):
                        for kb in range(4):
                            nc.tensor.matmul(
                                ps[:, ts(ih, 512)],
                                w_sb[kb][:, ts(eb, 128)],
                                x_sb[kb][:, ts(ih, 512)],
                                start=(kb == 0),
                                stop=(kb == 3),
                            )
                    if use_bias:
                        nc.scalar.activation(
                            out=o_sb[eb],
                            in_=ps,
                            func=Act.Copy,
                            bias=b_sb[:, eb : eb + 1],
                        )
                    else:
                        nc.vector.tensor_copy(out=o_sb[eb], in_=ps)

        # --- attention over row tiles ------------------------------------
        with (
            tc.tile_pool(name="s_ps", bufs=3, space="PSUM") as s_ps,
            tc.tile_pool(name="t_ps", bufs=1, space="PSUM") as t_ps,
            tc.tile_pool(name="av_ps", bufs=1, space="PSUM") as av_ps,
            tc.tile_pool(name="epool", bufs=5) as epool,
            tc.tile_pool(name="acc", bufs=3) as accp,
            tc.tile_pool(name="small", bufs=8) as smallp,
            tc.tile_pool(name="wmask", bufs=3) as wmaskp,
            tc.tile_pool(name="wloc", bufs=1) as wlocp,
            tc.tile_pool(name="wt", bufs=6) as wtp,
            tc.tile_pool(name="outp", bufs=1) as outp,
        ):
            for t in range(T if phase != "proj" else 0):
                w0 = _win_start(t)
                zt = smallp.tile([128, H], f32, tag="z", name="zt")
                rt = smallp.tile([128, H], f32, tag="r", name="rt")
                w_acc = accp.tile([128, WIN], f32, tag="wacc", name="w_acc")
                if phase in ("exp", "wacc"):
                    for h in range(H):
                        hp, sub = h // 2, h % 2
                        s_h = s_ps.tile([128, L], f32, tag="s", name="s_h")
                        for jh in range(2):
                            nc.tensor.matmul(
                                s_h[:, ts(jh, 512)],
                                qTp_sb[hp][ts(sub, 64), ts(t, 128)],
                                kTp_sb[hp][ts(sub, 64), ts(jh, 512)],
                                start=True,
                                stop=True,
                            )
                        e_h = epool.tile([128, L], f32, tag="e", name="e_h")
                        nc.scalar.activation(
                            out=e_h,
                            in_=s_h,
                            func=Act.Exp,
                            scale=0.125,
                            accum_out=zt[:, h : h + 1],
                        )
                        if phase == "wacc":
                            nc.vector.reciprocal(
                                out=rt[:, h : h + 1], in_=zt[:, h : h + 1]
                            )
                            ew = e_h[:, w0 : w0 + WIN]
                            if h == 0:
                                nc.vector.tensor_scalar(
                                    out=w_acc, in0=ew, scalar1=rt[:, h : h + 1],
                                    scalar2=None, op0=Alu.mult,
                                )
                            else:
                                nc.vector.scalar_tensor_tensor(
                                    out=w_acc, in0=ew, scalar=rt[:, h : h + 1],
                                    in1=w_acc, op0=Alu.mult, op1=Alu.add,
                                )
                        elif h == 0:
                            wloc_t = wlocp.tile(
                                [128, WIN], f32, tag=f"wl{t}", name=f"wloc_{t}"
                            )
                            nc.vector.tensor_copy(out=wloc_t, in_=e_h[:, 0:WIN])
                            nc.sync.dma_start(out=wband_d[t], in_=wloc_t)
                    if phase == "wacc":
                        wloc_t = wlocp.tile(
                            [128, WIN], f32, tag=f"wl{t}", name=f"wloc_{t}"
                        )
                        nc.vector.tensor_copy(out=wloc_t, in_=w_acc)
                        nc.sync.dma_start(out=wband_d[t], in_=wloc_t)
                    continue
                if phase in ("mm", "mm0"):
                    for h in range(H if phase == "mm" else 1):
                        hp, sub = h // 2, h % 2
                        s_h = s_ps.tile([128, L], f32, tag="s", name="s_h")
                        for jh in range(2):
                            nc.tensor.matmul(
                                s_h[:, ts(jh, 512)],
                                qTp_sb[hp][ts(sub, 64), ts(t, 128)],
                                kTp_sb[hp][ts(sub, 64), ts(jh, 512)],
                                start=True,
                                stop=True,
                            )
                        if h == 0:
                            wloc_t = wlocp.tile(
                                [128, WIN], f32, tag=f"wl{t}", name=f"wloc_{t}"
                            )
                            nc.vector.tensor_copy(out=wloc_t, in_=s_h[:, 0:WIN])
                            nc.sync.dma_start(out=wband_d[t], in_=wloc_t)
                    continue
                for h in range(H):
                    hp, sub = h // 2, h % 2
                    s_h = s_ps.tile([128, L], f32, tag="s", name="s_h")
                    lhsT = qTp_sb[hp][ts(sub, 64), ts(t, 128)]
                    for jh in range(2):
                        nc.tensor.matmul(
                            s_h[:, ts(jh, 512)],
                            lhsT,
                            kTp_sb[hp][ts(sub, 64), ts(jh, 512)],
                            start=True,
                            stop=True,
                        )
                    e_h = epool.tile([128, L], f32, tag="e", name="e_h")
                    nc.scalar.activation(
                        out=e_h,
                        in_=s_h,
                        func=Act.Exp,
                        scale=0.125,
                        accum_out=zt[:, h : h + 1],
                    )
                    nc.vector.reciprocal(
                        out=rt[:, h : h + 1], in_=zt[:, h : h + 1]
                    )
                    ew = e_h[:, w0 : w0 + WIN]
                    if h == 0:
                        nc.vector.tensor_scalar(
                            out=w_acc,
                            in0=ew,
                            scalar1=rt[:, h : h + 1],
                            scalar2=None,
                            op0=Alu.mult,
                        )
                    else:
                        nc.vector.scalar_tensor_tensor(
                            out=w_acc,
                            in0=ew,
                            scalar=rt[:, h : h + 1],
                            in1=w_acc,
                            op0=Alu.mult,
                            op1=Alu.add,
                        )

                # band mask + band sum
                w_masked = wmaskp.tile([128, WIN], f32, tag="wm", name="w_masked")
                bs = smallp.tile([128, 1], f32, tag="bs", name="bs")
                nc.vector.tensor_mul(w_masked, w_acc, mask_sb[_mask_sel(t)])
                nc.vector.tensor_reduce(
                    out=bs, in_=w_masked, axis=mybir.AxisListType.X, op=Alu.add
                )
                bse = smallp.tile([128, 1], f32, tag="bse", name="bse")
                nc.vector.tensor_scalar(
                    out=bse, in0=bs, scalar1=8e-6, scalar2=None, op0=Alu.add
                )
                rb = smallp.tile([128, 1], f32, tag="rb", name="rb")
                nc.vector.reciprocal(out=rb, in_=bse)

                # w_loc output tile (fp32, exact zeros off-band); scaled
                # before the transpose so the AV result needs no rescale
                wloc_t = wlocp.tile(
                    [128, WIN], f32, tag=f"wl{t}", name=f"wloc_{t}"
                )
                nc.vector.tensor_scalar(
                    out=wloc_t, in0=w_masked, scalar1=rb, scalar2=None, op0=Alu.mult
                )
                nc.sync.dma_start(out=wband_d[t], in_=wloc_t)

                if phase == "scores":
                    continue
                # out tile: (w_loc @ v_window)
                wt_sb = []
                for b in range(3):
                    tp = t_ps.tile([128, 128], f32, tag="tp", name="tp")
                    nc.tensor.transpose(tp, wloc_t[:, ts(b, 128)], ident_sb)
                    wt_b = wtp.tile([128, 128], bf16, tag="wt", name="wt_b")
                    nc.vector.tensor_copy(out=wt_b, in_=tp)
                    wt_sb.append(wt_b)
                av = av_ps.tile([128, E], f32, tag="av", name="av")
                for b in range(3):
                    nc.tensor.matmul(
                        av,
                        wt_sb[b],
                        v_sb[w0 // 128 + b],
                        start=(b == 0),
                        stop=(b == 2),
                    )
                out_t = outp.tile([128, E], f32, tag=f"ot{t}", name=f"out_{t}")
                nc.vector.tensor_copy(out=out_t, in_=av)
                nc.sync.dma_start(out=out_d[ts(t, 128), :], in_=out_t)

    nc.compile()
    return nc


def _get_program(use_bias: bool = False, phase: str = "full"):
    key = ("nc", use_bias, phase)
    if key not in _CACHE:
        _CACHE[key] = _build_program(use_bias, phase)
    return _CACHE[key]


def kernel(query, key, value, Wq, Wk, Wv, bq, bk, bv):
    from concourse.bass_utils import run_bass_kernel_spmd

    query = np.asarray(query, dtype=np.float32)
    key = np.asarray(key, dtype=np.float32)
    value = np.asarray(value, dtype=np.float32)
    Wq = np.asarray(Wq, dtype=np.float32)
    Wk = np.asarray(Wk, dtype=np.float32)
    bq = np.asarray(bq, dtype=np.float32)
    bk = np.asarray(bk, dtype=np.float32)

    use_bias = bool(np.any(bq) or np.any(bk))
    nc = _get_program(use_bias)

    wqT = np.ascontiguousarray(Wq.T).astype(BF16)
    wkT = np.ascontiguousarray(Wk.T).astype(BF16)
    bq_t = np.ascontiguousarray(bq.reshape(4, 128).T)
    bk_t = np.ascontiguousarray(bk.reshape(4, 128).T)
    masks = _make_masks()
    ident = np.eye(128, dtype=np.float32)

    in_maps = []
    for n in range(NCORES):
        in_maps.append(
            {
                "qT": np.ascontiguousarray(query[:, n, :].T).astype(BF16),
                "kT": np.ascontiguousarray(key[:, n, :].T).astype(BF16),
                "v": np.ascontiguousarray(value[:, n, :]).astype(BF16),
                "wqT": wqT,
                "wkT": wkT,
                "bq": bq_t,
                "bk": bk_t,
                "masks": masks,
                "ident": ident,
            }
        )

    res = run_bass_kernel_spmd(nc, in_maps, list(range(NCORES)))
    results = res.results

    out = np.stack(
        [np.asarray(results[n]["out"], dtype=np.float32) for n in range(NCORES)],
        axis=1,
    )
    w_loc = np.zeros((N, L, L), dtype=np.float32)
    for n in range(NCORES):
        wband = np.asarray(results[n]["wband"], dtype=np.float32)
        for t in range(T):
            w0 = _win_start(t)
            w_loc[n, 128 * t : 128 * (t + 1), w0 : w0 + WIN] = wband[t]
    return out, w_loc
